# revision 5
# baseline (speedup 1.0000x reference)
# Paged sparse attention (GQA, block-masked new tokens) on 8 TRN2 NeuronCores.
#
# Sharding: tensor-parallel over the 8 KV heads (one KV head + its 4 Q heads
# per core). Every core sees all 8 sequences, so the compiled schedule
# (derived from page_tables/context_lens, identical across cores) is SPMD.
#
# Per (seq b, core n):
#   K^T [d=128, T] and Q^T [d=128, sg=1024] loaded via DMA-xbar transposed
#   loads (bf16). Scores S[sg, t] = Q^T.T @ K^T computed per 128-row q-tile
#   into PSUM; with sg = s*4+g ordering, the block-causal mask reduces to a
#   per-q-tile prefix length Tq = ctxp + 32*(i+1), so only the partial-page
#   columns [ctx, ctxp) need masking. exp runs on ACT with the softmax scale
#   folded in and accum_out producing the denominator for free (no max
#   subtraction needed: scaled scores are ~N(0,1)). Probs (bf16) are
#   transposed via DMA-xbar into P^T tiles; PV runs V-stationary producing
#   OUT^T [d, sg] accumulated over t-blocks in PSUM; OUT^T is transposed back
#   via DMA-xbar and scaled by 1/denom on the way out.

import sys

sys.path.insert(0, "/opt/trn_rl_repo")

import math

import ml_dtypes
import numpy as np

B = 8
S = 256
NUM_HEADS = 32
NUM_KV_HEADS = 8
G = NUM_HEADS // NUM_KV_HEADS  # 4
HD = 128
PAGE = 16
BLOCK = 32
MAX_PAGES = 128
C = MAX_PAGES * PAGE  # 2048
SCALE = 0.08838834764831845
SG = S * G  # 1024 q rows per (seq, kv head)
TMAX = C + S  # 2304
NTMAX = TMAX // 128  # 18

NEG = -1e30


def _schedule(page_tables: np.ndarray, context_lens: np.ndarray):
    """Per-seq schedule baked into the compiled kernel (same on all cores)."""
    seqs = []
    for b in range(B):
        ctx = int(context_lens[b])
        npg = (ctx + PAGE - 1) // PAGE
        ctxp = npg * PAGE
        pages = [int(p) for p in page_tables[b, :npg]]
        # maximal consecutive-page runs -> (start_page, num_pages)
        runs = []
        for p in pages:
            if runs and runs[-1][0] + runs[-1][1] == p:
                runs[-1][1] += 1
            else:
                runs.append([p, 1])
        ttot = ctxp + S
        seqs.append(
            dict(
                ctx=ctx,
                ctxp=ctxp,
                runs=runs,
                ttot=ttot,
                ntb=(ttot + 127) // 128,
                tq=[ctxp + BLOCK * (i + 1) for i in range(SG // 128)],
            )
        )
    return seqs


def _build(nc, seqs):
    import concourse.bass as bass  # noqa: F401
    import concourse.mybir as mybir
    import concourse.tile as tile

    bf16 = mybir.dt.bfloat16
    f32 = mybir.dt.float32

    qh = nc.dram_tensor("qh", [B * S, G * HD], bf16, kind="ExternalInput").ap()
    kh = nc.dram_tensor("kh", [B * S, HD], bf16, kind="ExternalInput").ap()
    vh = nc.dram_tensor("vh", [B * S, HD], bf16, kind="ExternalInput").ap()
    kch = nc.dram_tensor("kch", [MAX_PAGES * B * PAGE, HD], bf16, kind="ExternalInput").ap()
    vch = nc.dram_tensor("vch", [MAX_PAGES * B * PAGE, HD], bf16, kind="ExternalInput").ap()
    outh = nc.dram_tensor("outh", [B * S, G * HD], f32, kind="ExternalOutput").ap()

    # q viewed per seq as [sg=(s,g), d]; contiguous because each q row holds
    # the 4 grouped heads back to back.
    qv = qh.rearrange("(b s) (g d) -> b (s g) d", b=B, d=HD)
    outv = outh.rearrange("(b s) (g d) -> b (s g) d", b=B, d=HD)

    NQT = SG // 128  # 8 q-tiles per seq
    C0 = 1024  # psum score chunk split

    with tile.TileContext(nc) as tc:
        with (
            tc.tile_pool(name="kt", bufs=2) as kt_pool,
            tc.tile_pool(name="vt", bufs=2) as v_pool,
            tc.tile_pool(name="qt", bufs=2) as qt_pool,
            tc.tile_pool(name="pp", bufs=3) as p_pool,
            tc.tile_pool(name="pt", bufs=2) as pt_pool,
            tc.tile_pool(name="st", bufs=24) as stat_pool,
            tc.tile_pool(name="ot", bufs=2) as out_pool,
            tc.tile_pool(name="of", bufs=4) as of_pool,
            tc.tile_pool(name="ps_s", bufs=1, space="PSUM") as psum_s,
            tc.tile_pool(name="ps_o", bufs=1, space="PSUM") as psum_o,
        ):
            for b in range(B):
                sq = seqs[b]
                ctx, ctxp, ttot, ntb = sq["ctx"], sq["ctxp"], sq["ttot"], sq["ntb"]

                # ---- K^T via transposed loads ----
                kt = kt_pool.tile([128, TMAX], bf16, tag="kt")
                col = 0
                for start, n in sq["runs"]:
                    off = 0
                    while off < n:
                        cn = min(n - off, 16)  # <=256 rows per call
                        rows = cn * PAGE
                        nc.sync.dma_start_transpose(
                            kt[:, col : col + rows],
                            kch[(start + off) * PAGE : (start + off) * PAGE + rows, :],
                        )
                        col += rows
                        off += cn
                assert col == ctxp
                nc.sync.dma_start_transpose(
                    kt[:, ctxp : ctxp + S], kh[b * S : (b + 1) * S, :]
                )

                # ---- V natural [t within block, tb, d] ----
                vt = v_pool.tile([128, NTMAX, HD], bf16, tag="vt")
                if ttot % 128:
                    # zero the padded tail of the last t-block before loads
                    # (engine ops need 32-aligned partition bases, so clear
                    # the whole block and let the loads overwrite)
                    nc.vector.memset(vt[:, ntb - 1, :], 0.0)

                col = 0
                for start, n in sq["runs"]:
                    rows = n * PAGE
                    t0 = col
                    src = start * PAGE
                    while rows > 0:
                        tb = t0 // 128
                        p0 = t0 % 128
                        seg = min(rows, 128 - p0)
                        nc.sync.dma_start(
                            vt[p0 : p0 + seg, tb, :], vch[src : src + seg, :]
                        )
                        t0 += seg
                        src += seg
                        rows -= seg
                    col += n * PAGE
                # new tokens
                rows, t0, src = S, ctxp, b * S
                while rows > 0:
                    tb = t0 // 128
                    p0 = t0 % 128
                    seg = min(rows, 128 - p0)
                    nc.sync.dma_start(vt[p0 : p0 + seg, tb, :], vh[src : src + seg, :])
                    t0 += seg
                    src += seg
                    rows -= seg
                # ---- Q^T via transposed loads (4 chunks of 256 rows) ----
                qt = qt_pool.tile([128, SG], bf16, tag="qt")
                for c in range(4):
                    nc.sync.dma_start_transpose(
                        qt[:, c * 256 : (c + 1) * 256],
                        qv[b, c * 256 : (c + 1) * 256, :],
                    )

                # ---- per q-tile: scores, exp, P^T ----
                ptt = pt_pool.tile([128, NQT, NTMAX, HD], bf16, tag="pt")
                invs = []
                for i in range(NQT):
                    tq = sq["tq"][i]
                    ntq = (tq + 127) // 128
                    c0len = min(tq, C0)
                    c1len = tq - c0len
                    s_a = psum_s.tile([128, C0], f32, tag="sa")
                    chunks = [(s_a, 0, c0len)]
                    if c1len:
                        s_b = psum_s.tile([128, TMAX - C0], f32, tag="sb")
                        chunks.append((s_b, C0, c1len))

                    for s_t, base, ln in chunks:
                        for c0 in range(0, ln, 512):
                            cn = min(512, ln - c0)
                            nc.tensor.matmul(
                                s_t[:, c0 : c0 + cn],
                                lhsT=qt[:, i * 128 : (i + 1) * 128],
                                rhs=kt[:, base + c0 : base + c0 + cn],
                                start=True,
                                stop=True,
                            )
                        # partial-page mask: scores for t in [ctx, ctxp) -> -inf
                        m0, m1 = max(ctx, base), min(ctxp, base + ln)
                        if m0 < m1:
                            nc.vector.memset(s_t[:, m0 - base : m1 - base], NEG)

                    p_t = p_pool.tile([128, TMAX], bf16, tag="p")
                    denom = stat_pool.tile([128, 1], f32, tag="dn")
                    nc.scalar.activation(
                        out=p_t[:, :c0len],
                        in_=s_a[:, :c0len],
                        func=mybir.ActivationFunctionType.Exp,
                        scale=SCALE,
                        accum_out=denom,
                    )
                    if c1len:
                        denom2 = stat_pool.tile([128, 1], f32, tag="dn2")
                        nc.scalar.activation(
                            out=p_t[:, C0 : C0 + c1len],
                            in_=s_b[:, :c1len],
                            func=mybir.ActivationFunctionType.Exp,
                            scale=SCALE,
                            accum_out=denom2,
                        )
                        nc.vector.tensor_add(denom, denom, denom2)
                    if tq % 128:
                        nc.vector.memset(p_t[:, tq : ntq * 128], 0.0)
                    inv = stat_pool.tile([128, 1], f32, tag="inv")
                    nc.vector.reciprocal(inv, denom)
                    invs.append(inv)
                    # P^T: [128 sg, ntq*128 t] -> [128 t, ntq, 128 sg]
                    nc.sync.dma_start_transpose(
                        ptt[:, i, :ntq, :], p_t[:, : ntq * 128]
                    )

                # ---- PV: OUT^T[d, sg] += V[tb].T-stationary @ P^T[tb] ----
                outt = psum_o.tile([128, SG], f32, tag="outt")
                for tb in range(ntb):
                    qmin = next(
                        i for i in range(NQT) if sq["tq"][i] > tb * 128
                    )
                    for g0, g1 in ((0, 4), (4, 8)):
                        lo = max(qmin, g0)
                        if lo >= g1:
                            continue
                        nc.tensor.matmul(
                            outt[:, lo * 128 : g1 * 128],
                            lhsT=vt[:, tb, :],
                            rhs=ptt[:, lo:g1, tb, :],
                            start=(tb == 0),
                            stop=(tb == ntb - 1),
                        )

                # ---- endgame: OUT^T -> bf16 -> transpose -> scale -> HBM ----
                outt_sb = out_pool.tile([128, SG], bf16, tag="otsb")
                nc.vector.tensor_copy(outt_sb, outt)
                out_sb = out_pool.tile([128, NQT, HD], bf16, tag="osb")
                nc.sync.dma_start_transpose(out_sb, outt_sb)
                for i in range(NQT):
                    of = of_pool.tile([128, HD], f32, tag="of")
                    nc.vector.tensor_scalar_mul(of, out_sb[:, i, :], invs[i])
                    nc.sync.dma_start(
                        outv[b, i * 128 : (i + 1) * 128, :], of
                    )
    return nc


def _compile(seqs):
    import concourse.bacc as bacc

    nc = bacc.Bacc(
        "TRN2",
        target_bir_lowering=False,
        debug=False,
        enable_asserts=False,
        num_devices=8,
    )
    _build(nc, seqs)
    nc.compile()
    return nc


def kernel(q, k, v, k_cache, v_cache, page_tables, context_lens, page_size, block_size, **_):
    from concourse import bass_utils

    q = np.asarray(q)
    k = np.asarray(k)
    v = np.asarray(v)
    k_cache = np.asarray(k_cache)
    v_cache = np.asarray(v_cache)
    page_tables = np.asarray(page_tables)
    context_lens = np.asarray(context_lens)
    assert int(page_size) == PAGE and int(block_size) == BLOCK
    assert q.shape == (B * S, NUM_HEADS * HD)
    assert page_tables.shape == (B, MAX_PAGES)

    seqs = _schedule(page_tables, context_lens)
    nc = _compile(seqs)

    bf = ml_dtypes.bfloat16
    # per-core head slices (cast to bf16 on host; device does all FLOPs)
    kcv = k_cache.reshape(MAX_PAGES * B * PAGE, NUM_KV_HEADS, HD)
    vcv = v_cache.reshape(MAX_PAGES * B * PAGE, NUM_KV_HEADS, HD)
    in_maps = []
    for n in range(NUM_KV_HEADS):
        in_maps.append(
            {
                "qh": np.ascontiguousarray(
                    q[:, n * G * HD : (n + 1) * G * HD]
                ).astype(bf),
                "kh": np.ascontiguousarray(k[:, n * HD : (n + 1) * HD]).astype(bf),
                "vh": np.ascontiguousarray(v[:, n * HD : (n + 1) * HD]).astype(bf),
                "kch": np.ascontiguousarray(kcv[:, n, :]).astype(bf),
                "vch": np.ascontiguousarray(vcv[:, n, :]).astype(bf),
            }
        )

    res = bass_utils.run_bass_kernel_spmd(
        nc, in_maps, core_ids=list(range(8))
    )
    global _last_results
    _last_results = res
    out = np.concatenate([r["outh"] for r in res.results], axis=1)
    return out.astype(np.float32)


_last_results = None


if __name__ == "__main__":
    pass


# revision 6
# speedup vs baseline: 1.4106x; 1.4106x over previous
# Paged sparse attention (GQA, block-masked new tokens) on 8 TRN2 NeuronCores.
#
# Sharding: tensor-parallel over the 8 KV heads (one KV head + its 4 Q heads
# per core). Every core sees all 8 sequences, so the compiled schedule
# (derived from page_tables/context_lens, identical across cores) is SPMD.
#
# Per (seq b, core n):
#   K^T [d=128, T] and Q^T [d=128, sg=1024] loaded via DMA-xbar transposed
#   loads (bf16). Scores S[sg, t] = Q^T.T @ K^T computed per 128-row q-tile
#   into PSUM; with sg = s*4+g ordering, the block-causal mask reduces to a
#   per-q-tile prefix length Tq = ctxp + 32*(i+1), so only the partial-page
#   columns [ctx, ctxp) need masking. exp runs on ACT with the softmax scale
#   folded in and accum_out producing the denominator for free (no max
#   subtraction needed: scaled scores are ~N(0,1)). Probs (bf16) are
#   transposed via one DMA-xbar call per seq into P^T tiles; PV runs
#   V-stationary producing OUT^T [d, sg] accumulated over t-blocks in PSUM;
#   OUT^T is transposed back via DMA-xbar and scaled by 1/denom on the way
#   out. DMA calls are merged aggressively: the sequencer-side issue cost of
#   a DMA (~0.6-1.4us) is per-call, independent of size.

import sys

sys.path.insert(0, "/opt/trn_rl_repo")

import ml_dtypes
import numpy as np

B = 8
S = 256
NUM_HEADS = 32
NUM_KV_HEADS = 8
G = NUM_HEADS // NUM_KV_HEADS  # 4
HD = 128
PAGE = 16
BLOCK = 32
MAX_PAGES = 128
C = MAX_PAGES * PAGE  # 2048
SCALE = 0.08838834764831845
SG = S * G  # 1024 q rows per (seq, kv head)
TMAX = C + S  # 2304
NQT = SG // 128  # 8 q-tiles per seq

NEG = -1e30


def _schedule(page_tables: np.ndarray, context_lens: np.ndarray):
    """Per-seq schedule baked into the compiled kernel (same on all cores)."""
    seqs = []
    for b in range(B):
        ctx = int(context_lens[b])
        npg = (ctx + PAGE - 1) // PAGE
        ctxp = npg * PAGE
        pages = [int(p) for p in page_tables[b, :npg]]
        runs = []  # maximal consecutive-page runs -> [start_page, num_pages]
        for p in pages:
            if runs and runs[-1][0] + runs[-1][1] == p:
                runs[-1][1] += 1
            else:
                runs.append([p, 1])
        ttot = ctxp + S
        seqs.append(
            dict(
                ctx=ctx,
                ctxp=ctxp,
                runs=runs,
                ttot=ttot,
                ntb=(ttot + 127) // 128,
                tq=[ctxp + BLOCK * (i + 1) for i in range(NQT)],
            )
        )
    return seqs


def _build(nc, seqs):
    import concourse.mybir as mybir
    import concourse.tile as tile

    bf16 = mybir.dt.bfloat16
    f32 = mybir.dt.float32

    qh = nc.dram_tensor("qh", [B * S, G * HD], bf16, kind="ExternalInput").ap()
    kh = nc.dram_tensor("kh", [B * S, HD], bf16, kind="ExternalInput").ap()
    vh = nc.dram_tensor("vh", [B * S, HD], bf16, kind="ExternalInput").ap()
    kch = nc.dram_tensor("kch", [MAX_PAGES * B * PAGE, HD], bf16, kind="ExternalInput").ap()
    vch = nc.dram_tensor("vch", [MAX_PAGES * B * PAGE, HD], bf16, kind="ExternalInput").ap()
    outh = nc.dram_tensor("outh", [B * S, G * HD], f32, kind="ExternalOutput").ap()

    # q viewed per seq as [sg=(s,g), d]; contiguous because each q row holds
    # the 4 grouped heads back to back. Same layout for the output.
    qv = qh.rearrange("(b s) (g d) -> b (s g) d", b=B, d=HD)
    outv = outh.rearrange("(b s) (g d) -> b (s g) d", b=B, d=HD)

    with tile.TileContext(nc) as tc:
        with (
            tc.tile_pool(name="kt", bufs=2) as kt_pool,
            tc.tile_pool(name="vt", bufs=2) as v_pool,
            tc.tile_pool(name="qt", bufs=2) as qt_pool,
            tc.tile_pool(name="pp", bufs=2) as p_pool,
            tc.tile_pool(name="pt", bufs=2) as pt_pool,
            tc.tile_pool(name="st", bufs=4) as stat_pool,
            tc.tile_pool(name="ot", bufs=2) as out_pool,
            tc.tile_pool(name="ps_s", bufs=1, space="PSUM") as psum_s,
            tc.tile_pool(name="ps_o", bufs=1, space="PSUM") as psum_o,
        ):
            for b in range(B):
                sq = seqs[b]
                ctx, ctxp, ttot, ntb = sq["ctx"], sq["ctxp"], sq["ttot"], sq["ntb"]

                # ---- K^T via transposed loads (one call per page-run) ----
                kt = kt_pool.tile([128, TMAX], bf16, tag="kt")
                col = 0
                for start, n in sq["runs"]:
                    nc.sync.dma_start_transpose(
                        kt[:, col : col + n * PAGE],
                        kch[start * PAGE : (start + n) * PAGE, :],
                    )
                    col += n * PAGE
                assert col == ctxp
                nc.sync.dma_start_transpose(
                    kt[:, ctxp : ctxp + S], kh[b * S : (b + 1) * S, :]
                )

                # ---- V natural [t%128, tb, d]; big rearranged DMAs ----
                vt = v_pool.tile([128, ntb, HD], bf16, tag="vt")
                if ttot % 128:
                    # zero the padded tail of the last t-block before loads
                    nc.vector.memset(vt[:, ntb - 1, :], 0.0)

                def load_v_rows(t0, nrows, src, src_row0):
                    """copy src rows [src_row0, +nrows) to packed t [t0, ...)"""
                    # leading partial block
                    while nrows > 0 and t0 % 128:
                        seg = min(nrows, 128 - t0 % 128)
                        nc.gpsimd.dma_start(
                            vt[t0 % 128 : t0 % 128 + seg, t0 // 128, :],
                            src[src_row0 : src_row0 + seg, :],
                        )
                        t0 += seg
                        src_row0 += seg
                        nrows -= seg
                    # bulk full blocks in one call
                    nfull = (nrows // 128) * 128
                    if nfull:
                        nc.gpsimd.dma_start(
                            vt[:, t0 // 128 : t0 // 128 + nfull // 128, :],
                            src[src_row0 : src_row0 + nfull, :].rearrange(
                                "(tb p) d -> p tb d", p=128
                            ),
                        )
                        t0 += nfull
                        src_row0 += nfull
                        nrows -= nfull
                    if nrows:
                        nc.gpsimd.dma_start(
                            vt[: nrows, t0 // 128, :],
                            src[src_row0 : src_row0 + nrows, :],
                        )

                col = 0
                for start, n in sq["runs"]:
                    load_v_rows(col, n * PAGE, vch, start * PAGE)
                    col += n * PAGE
                load_v_rows(ctxp, S, vh, b * S)

                # ---- Q^T via one transposed load ----
                qt = qt_pool.tile([128, SG], bf16, tag="qt")
                nc.sync.dma_start_transpose(qt, qv[b])

                # ---- per q-tile: scores -> exp(+denom) into P_seq ----
                p_seq = p_pool.tile([128, NQT, ntb * 128], bf16, tag="p")
                denoms = stat_pool.tile([128, NQT], f32, tag="dn")
                for i in range(NQT):
                    tq = sq["tq"][i]
                    ntq = (tq + 127) // 128
                    s_ps = psum_s.tile([128, TMAX], f32, tag="s")
                    for c0 in range(0, tq, 512):
                        cn = min(512, tq - c0)
                        nc.tensor.matmul(
                            s_ps[:, c0 : c0 + cn],
                            lhsT=qt[:, i * 128 : (i + 1) * 128],
                            rhs=kt[:, c0 : c0 + cn],
                            start=True,
                            stop=True,
                        )
                    if ctx < ctxp:
                        # partial-page mask: t in [ctx, ctxp) -> -inf
                        nc.vector.memset(s_ps[:, ctx:ctxp], NEG)
                    nc.scalar.activation(
                        out=p_seq[:, i, :tq],
                        in_=s_ps[:, :tq],
                        func=mybir.ActivationFunctionType.Exp,
                        scale=SCALE,
                        accum_out=denoms[:, i : i + 1],
                    )
                    if tq % 128:
                        nc.vector.memset(p_seq[:, i, tq : ntq * 128], 0.0)

                # ---- P^T: one xbar call per seq ----
                # [128 sg, (i,tb)*128 t] -> [128 t, (i,tb), 128 sg]
                ptt = pt_pool.tile([128, NQT, ntb, 128], bf16, tag="pt")
                nc.sync.dma_start_transpose(ptt, p_seq)

                # ---- PV: OUT^T[d, sg] += V[tb] (stationary) @ P^T[tb] ----
                outt = psum_o.tile([128, SG], f32, tag="outt")
                last_tb = [0, 0]
                for tb in range(ntb):
                    qmin = next(i for i in range(NQT) if sq["tq"][i] > tb * 128)
                    for ci, (g0, g1) in enumerate(((0, 4), (4, 8))):
                        if max(qmin, g0) < g1:
                            last_tb[ci] = tb
                for tb in range(ntb):
                    qmin = next(i for i in range(NQT) if sq["tq"][i] > tb * 128)
                    for ci, (g0, g1) in enumerate(((0, 4), (4, 8))):
                        lo = max(qmin, g0)
                        if lo >= g1:
                            continue
                        nc.tensor.matmul(
                            outt[:, lo * 128 : g1 * 128],
                            lhsT=vt[:, tb, :],
                            rhs=ptt[:, lo:g1, tb, :],
                            start=(tb == 0),
                            stop=(tb == last_tb[ci]),
                        )

                # ---- endgame: OUT^T -> bf16 -> transpose -> scale -> HBM ----
                outt_sb = out_pool.tile([128, SG], bf16, tag="otsb")
                nc.vector.tensor_copy(outt_sb, outt)
                out_sb = out_pool.tile([128, NQT, HD], bf16, tag="osb")
                nc.sync.dma_start_transpose(out_sb, outt_sb)
                invs = stat_pool.tile([128, NQT], f32, tag="inv")
                nc.vector.reciprocal(invs, denoms)
                of = out_pool.tile([128, NQT, HD], f32, tag="of")
                nc.vector.tensor_tensor(
                    of,
                    out_sb,
                    invs[:, :, None].to_broadcast([128, NQT, HD]),
                    mybir.AluOpType.mult,
                )
                nc.gpsimd.dma_start(
                    outv[b].rearrange("(i p) d -> p i d", p=128), of
                )
    return nc


def _compile(seqs):
    import concourse.bacc as bacc

    nc = bacc.Bacc(
        "TRN2",
        target_bir_lowering=False,
        debug=False,
        enable_asserts=False,
        num_devices=8,
    )
    _build(nc, seqs)
    nc.compile()
    return nc


def kernel(q, k, v, k_cache, v_cache, page_tables, context_lens, page_size, block_size, **_):
    from concourse import bass_utils

    q = np.asarray(q)
    k = np.asarray(k)
    v = np.asarray(v)
    k_cache = np.asarray(k_cache)
    v_cache = np.asarray(v_cache)
    page_tables = np.asarray(page_tables)
    context_lens = np.asarray(context_lens)
    assert int(page_size) == PAGE and int(block_size) == BLOCK
    assert q.shape == (B * S, NUM_HEADS * HD)
    assert page_tables.shape == (B, MAX_PAGES)

    seqs = _schedule(page_tables, context_lens)
    nc = _compile(seqs)

    bf = ml_dtypes.bfloat16
    kcv = k_cache.reshape(MAX_PAGES * B * PAGE, NUM_KV_HEADS, HD)
    vcv = v_cache.reshape(MAX_PAGES * B * PAGE, NUM_KV_HEADS, HD)
    in_maps = []
    for n in range(NUM_KV_HEADS):
        in_maps.append(
            {
                "qh": np.ascontiguousarray(
                    q[:, n * G * HD : (n + 1) * G * HD]
                ).astype(bf),
                "kh": np.ascontiguousarray(k[:, n * HD : (n + 1) * HD]).astype(bf),
                "vh": np.ascontiguousarray(v[:, n * HD : (n + 1) * HD]).astype(bf),
                "kch": np.ascontiguousarray(kcv[:, n, :]).astype(bf),
                "vch": np.ascontiguousarray(vcv[:, n, :]).astype(bf),
            }
        )

    res = bass_utils.run_bass_kernel_spmd(nc, in_maps, core_ids=list(range(8)))
    global _last_results
    _last_results = res
    out = np.concatenate([r["outh"] for r in res.results], axis=1)
    return out.astype(np.float32)


_last_results = None


# revision 7
# speedup vs baseline: 2.4093x; 1.7080x over previous
# Paged sparse attention (GQA, block-masked new tokens) on 8 TRN2 NeuronCores.
#
# Sharding: tensor-parallel over the 8 KV heads (one KV head + its 4 Q heads
# per core). Every core sees all 8 sequences, so the compiled schedule
# (derived from page_tables/context_lens, identical across cores) is SPMD.
#
# Orientation: scores are computed TRANSPOSED (S^T[t, sg] per 128-row
# t-block, K^T-stationary, Q^T-moving), so the exp (ACT) writes P^T directly
# in the layout the PV matmul consumes — no probability transposes anywhere.
# Masking folds into the exp's per-partition bias (host-precomputed -1e30
# rows for the partial page / 32-alignment gap / tail pad). The softmax
# denominator comes from an extra matmul with an all-ones stationary matrix,
# which leaves the per-sg denominator replicated across all 128 PSUM
# partitions — the normalization is then a single fused
# (OUT^T * 1/denom -> bf16) DVE pass, transposed back to [sg, d] by one
# DMA-xbar call per sequence.
#
# The block-causal mask for new tokens reduces (with sg = s*4+g ordering) to
# a suffix of valid sg columns per t-block (plus a small intra-block
# staircase zeroed on the bf16 P^T), so invalid regions are simply never
# computed.

import sys

sys.path.insert(0, "/opt/trn_rl_repo")

import ml_dtypes
import numpy as np

B = 8
S = 256
NUM_HEADS = 32
NUM_KV_HEADS = 8
G = NUM_HEADS // NUM_KV_HEADS  # 4
HD = 128
PAGE = 16
BLOCK = 32
MAX_PAGES = 128
C = MAX_PAGES * PAGE  # 2048
SCALE = 0.08838834764831845
SG = S * G  # 1024 q rows per (seq, kv head)
TMAX = C + S + 32  # worst-case padded length
NTBMAX = (TMAX + 127) // 128
NQT = SG // 128  # 8 q-tiles per seq

NEG = -1e30


def _schedule(page_tables: np.ndarray, context_lens: np.ndarray):
    """Per-seq schedule baked into the compiled kernel (same on all cores)."""
    seqs = []
    for b in range(B):
        ctx = int(context_lens[b])
        npg = (ctx + PAGE - 1) // PAGE
        ctxp = npg * PAGE
        ctxp32 = ((ctxp + 31) // 32) * 32  # 32-align the new-token region
        pages = [int(p) for p in page_tables[b, :npg]]
        runs = []  # maximal consecutive-page runs -> [start_page, num_pages]
        for p in pages:
            if runs and runs[-1][0] + runs[-1][1] == p:
                runs[-1][1] += 1
            else:
                runs.append([p, 1])
        ttot = ctxp32 + S
        ntb = (ttot + 127) // 128
        tq = [ctxp32 + BLOCK * (i + 1) for i in range(NQT)]
        # first valid q-tile per t-block (valid sg columns = suffix)
        qmin = [next(i for i in range(NQT) if tq[i] > tb * 128) for tb in range(ntb)]
        seqs.append(
            dict(
                ctx=ctx,
                ctxp=ctxp,
                ctxp32=ctxp32,
                runs=runs,
                ttot=ttot,
                ntb=ntb,
                tq=tq,
                qmin=qmin,
            )
        )
    return seqs


def _masks(seqs):
    """Host-precomputed per-partition exp bias: [B, 128, NTBMAX] fp32.
    mask[b, p, tb] is added (post-scale) to scores of t-row tb*128+p:
    0 for valid rows, -1e30 for masked rows (partial page, 32-align gap,
    padded tail)."""
    m = np.zeros((B, 128, NTBMAX), np.float32)
    for b, sq in enumerate(seqs):
        valid = np.zeros((NTBMAX * 128,), bool)
        valid[: sq["ttot"]] = True
        valid[sq["ctx"] : sq["ctxp32"]] = False  # partial page + gap
        m[b][~valid.reshape(NTBMAX, 128).T] = NEG
    return m


def _build(nc, seqs):
    import concourse.mybir as mybir
    import concourse.tile as tile

    bf16 = mybir.dt.bfloat16
    f32 = mybir.dt.float32

    qh = nc.dram_tensor("qh", [B * S, G * HD], bf16, kind="ExternalInput").ap()
    kh = nc.dram_tensor("kh", [B * S, HD], bf16, kind="ExternalInput").ap()
    vh = nc.dram_tensor("vh", [B * S, HD], bf16, kind="ExternalInput").ap()
    kch = nc.dram_tensor("kch", [MAX_PAGES * B * PAGE, HD], bf16, kind="ExternalInput").ap()
    vch = nc.dram_tensor("vch", [MAX_PAGES * B * PAGE, HD], bf16, kind="ExternalInput").ap()
    mh = nc.dram_tensor("mh", [B, 128, NTBMAX], f32, kind="ExternalInput").ap()
    zz = nc.dram_tensor("zz", [32, HD], bf16, kind="ExternalInput").ap()
    outh = nc.dram_tensor("outh", [B * S, G * HD], f32, kind="ExternalOutput").ap()

    # q viewed per seq as [sg=(s,g), d]; contiguous because each q row holds
    # the 4 grouped heads back to back. Same layout for the output.
    qv = qh.rearrange("(b s) (g d) -> b (s g) d", b=B, d=HD)
    outv = outh.rearrange("(b s) (g d) -> b (s g) d", b=B, d=HD)

    with tile.TileContext(nc) as tc:
        with (
            tc.tile_pool(name="cst", bufs=1) as const_pool,
            tc.tile_pool(name="kt", bufs=2) as kt_pool,
            tc.tile_pool(name="vt", bufs=2) as v_pool,
            tc.tile_pool(name="qt", bufs=2) as qt_pool,
            tc.tile_pool(name="pt", bufs=2) as pt_pool,
            tc.tile_pool(name="mk", bufs=2) as mask_pool,
            tc.tile_pool(name="ot", bufs=2) as out_pool,
            tc.tile_pool(name="ps_s", bufs=2, space="PSUM") as psum_s,
            tc.tile_pool(name="ps_o", bufs=1, space="PSUM") as psum_o,
            tc.tile_pool(name="ps_d", bufs=1, space="PSUM") as psum_d,
        ):
            ones_t = const_pool.tile([128, 128], bf16)
            nc.vector.memset(ones_t, 1.0)

            for b in range(B):
                sq = seqs[b]
                ctx, ctxp, ctxp32 = sq["ctx"], sq["ctxp"], sq["ctxp32"]
                ttot, ntb, tq, qmin = sq["ttot"], sq["ntb"], sq["tq"], sq["qmin"]

                # ---- K^T via transposed loads (one call per page-run) ----
                kt = kt_pool.tile([128, NTBMAX * 128], bf16, tag="kt")
                col = 0
                for start, n in sq["runs"]:
                    nc.sync.dma_start_transpose(
                        kt[:, col : col + n * PAGE],
                        kch[start * PAGE : (start + n) * PAGE, :],
                    )
                    col += n * PAGE
                assert col == ctxp
                if ctxp32 > ctxp:  # 32-align gap: zero K columns
                    nc.vector.memset(kt[:, ctxp:ctxp32], 0.0)
                nc.sync.dma_start_transpose(
                    kt[:, ctxp32 : ctxp32 + S], kh[b * S : (b + 1) * S, :]
                )
                if ntb * 128 > ttot:  # zero padded tail columns
                    nc.vector.memset(kt[:, ttot : ntb * 128], 0.0)

                # ---- V natural [t%128, tb, d]; big rearranged DMAs ----
                vt = v_pool.tile([128, NTBMAX, HD], bf16, tag="vt")
                if ttot % 128:
                    # zero last block before loads (NaN-safe padded tail)
                    nc.vector.memset(vt[:, ntb - 1, :], 0.0)

                def load_v_rows(t0, nrows, src, src_row0):
                    """copy src rows [src_row0, +nrows) to packed t [t0, ...)"""
                    while nrows > 0 and t0 % 128:
                        seg = min(nrows, 128 - t0 % 128)
                        nc.gpsimd.dma_start(
                            vt[t0 % 128 : t0 % 128 + seg, t0 // 128, :],
                            src[src_row0 : src_row0 + seg, :],
                        )
                        t0 += seg
                        src_row0 += seg
                        nrows -= seg
                    nfull = (nrows // 128) * 128
                    if nfull:
                        nc.gpsimd.dma_start(
                            vt[:, t0 // 128 : t0 // 128 + nfull // 128, :],
                            src[src_row0 : src_row0 + nfull, :].rearrange(
                                "(tb p) d -> p tb d", p=128
                            ),
                        )
                        t0 += nfull
                        src_row0 += nfull
                        nrows -= nfull
                    if nrows:
                        nc.gpsimd.dma_start(
                            vt[:nrows, t0 // 128, :],
                            src[src_row0 : src_row0 + nrows, :],
                        )

                col = 0
                for start, n in sq["runs"]:
                    load_v_rows(col, n * PAGE, vch, start * PAGE)
                    col += n * PAGE
                if ctxp32 > ctxp and ctxp // 128 != ntb - 1:
                    # NaN-safe zeros for the gap rows (unless already zeroed
                    # by the last-block memset)
                    load_v_rows(ctxp, ctxp32 - ctxp, zz, 0)
                load_v_rows(ctxp32, S, vh, b * S)

                # ---- Q^T via one transposed load ----
                qt = qt_pool.tile([128, SG], bf16, tag="qt")
                nc.sync.dma_start_transpose(qt, qv[b])

                # ---- exp bias mask [128, ntb] ----
                mask_sb = mask_pool.tile([128, NTBMAX], f32, tag="mk")
                nc.sync.dma_start(mask_sb, mh[b])

                # ---- per t-block: S^T = K^T.T @ Q^T -> exp -> P^T ----
                ptt = pt_pool.tile([128, NTBMAX, SG], bf16, tag="pt")
                for tb in range(ntb):
                    qm = qmin[tb]
                    s_ps = psum_s.tile([128, SG], f32, tag="s")
                    for c0, c1 in ((qm * 128, 512), (max(512, qm * 128), SG)):
                        if c0 >= c1:
                            continue
                        nc.tensor.matmul(
                            s_ps[:, c0:c1],
                            lhsT=kt[:, tb * 128 : (tb + 1) * 128],
                            rhs=qt[:, c0:c1],
                            start=True,
                            stop=True,
                        )
                    nc.scalar.activation(
                        out=ptt[:, tb, qm * 128 :],
                        in_=s_ps[:, qm * 128 :],
                        func=mybir.ActivationFunctionType.Exp,
                        scale=SCALE,
                        bias=mask_sb[:, tb : tb + 1],
                    )
                    # staircase: zero P^T rows of new-token blocks for
                    # earlier q-tiles inside this t-block's suffix
                    for r0 in range(0, 128, 32):
                        t0 = tb * 128 + r0
                        if t0 < ctxp32 or t0 >= ttot:
                            continue
                        blk = (t0 - ctxp32) // 32
                        if blk > qm:
                            nc.vector.memset(
                                ptt[r0 : r0 + 32, tb, qm * 128 : blk * 128], 0.0
                            )

                # ---- PV: OUT^T[d, sg] += V[tb] (stationary) @ P^T[tb] ----
                outt = psum_o.tile([128, SG], f32, tag="outt")
                chunks = ((0, 4), (4, 8))
                last_tb = [0, 0]
                for tb in range(ntb):
                    for ci, (g0, g1) in enumerate(chunks):
                        if max(qmin[tb], g0) < g1:
                            last_tb[ci] = tb
                for tb in range(ntb):
                    for ci, (g0, g1) in enumerate(chunks):
                        lo = max(qmin[tb], g0)
                        if lo >= g1:
                            continue
                        nc.tensor.matmul(
                            outt[:, lo * 128 : g1 * 128],
                            lhsT=vt[:, tb, :],
                            rhs=ptt[:, tb, lo * 128 : g1 * 128],
                            start=(tb == 0),
                            stop=(tb == last_tb[ci]),
                        )

                # ---- denominators: ones-matmul -> replicated column sums ----
                dent = psum_d.tile([128, SG], f32, tag="dent")
                for tb in range(ntb):
                    for ci, (g0, g1) in enumerate(chunks):
                        lo = max(qmin[tb], g0)
                        if lo >= g1:
                            continue
                        nc.tensor.matmul(
                            dent[:, lo * 128 : g1 * 128],
                            lhsT=ones_t,
                            rhs=ptt[:, tb, lo * 128 : g1 * 128],
                            start=(tb == 0),
                            stop=(tb == last_tb[ci]),
                        )

                # ---- endgame: OUT^T * (1/denom) -> bf16 -> transpose ----
                invt = out_pool.tile([128, SG], f32, tag="invt")
                nc.vector.reciprocal(invt, dent)
                outt_sb = out_pool.tile([128, SG], bf16, tag="otsb")
                nc.vector.tensor_mul(outt_sb, outt, invt)
                out_sb = out_pool.tile([128, NQT, HD], bf16, tag="osb")
                nc.sync.dma_start_transpose(out_sb, outt_sb)
                of = out_pool.tile([128, NQT, HD], f32, tag="of")
                nc.vector.tensor_copy(of, out_sb)
                nc.gpsimd.dma_start(
                    outv[b].rearrange("(i p) d -> p i d", p=128), of
                )
    return nc


def _compile(seqs):
    import concourse.bacc as bacc

    nc = bacc.Bacc(
        "TRN2",
        target_bir_lowering=False,
        debug=False,
        enable_asserts=False,
        num_devices=8,
    )
    _build(nc, seqs)
    nc.compile()
    return nc


def kernel(q, k, v, k_cache, v_cache, page_tables, context_lens, page_size, block_size, **_):
    from concourse import bass_utils

    q = np.asarray(q)
    k = np.asarray(k)
    v = np.asarray(v)
    k_cache = np.asarray(k_cache)
    v_cache = np.asarray(v_cache)
    page_tables = np.asarray(page_tables)
    context_lens = np.asarray(context_lens)
    assert int(page_size) == PAGE and int(block_size) == BLOCK
    assert q.shape == (B * S, NUM_HEADS * HD)
    assert page_tables.shape == (B, MAX_PAGES)

    seqs = _schedule(page_tables, context_lens)
    nc = _compile(seqs)

    bf = ml_dtypes.bfloat16
    masks = _masks(seqs)
    kcv = k_cache.reshape(MAX_PAGES * B * PAGE, NUM_KV_HEADS, HD)
    vcv = v_cache.reshape(MAX_PAGES * B * PAGE, NUM_KV_HEADS, HD)
    zz = np.zeros((32, HD), bf)
    in_maps = []
    for n in range(NUM_KV_HEADS):
        in_maps.append(
            {
                "qh": np.ascontiguousarray(
                    q[:, n * G * HD : (n + 1) * G * HD]
                ).astype(bf),
                "kh": np.ascontiguousarray(k[:, n * HD : (n + 1) * HD]).astype(bf),
                "vh": np.ascontiguousarray(v[:, n * HD : (n + 1) * HD]).astype(bf),
                "kch": np.ascontiguousarray(kcv[:, n, :]).astype(bf),
                "vch": np.ascontiguousarray(vcv[:, n, :]).astype(bf),
                "mh": masks,
                "zz": zz,
            }
        )

    res = bass_utils.run_bass_kernel_spmd(nc, in_maps, core_ids=list(range(8)))
    global _last_results
    _last_results = res
    out = np.concatenate([r["outh"] for r in res.results], axis=1)
    return out.astype(np.float32)


_last_results = None


# revision 8
# speedup vs baseline: 2.7049x; 1.1227x over previous
# Paged sparse attention (GQA, block-masked new tokens) on 8 TRN2 NeuronCores.
#
# Sharding: tensor-parallel over the 8 KV heads (one KV head + its 4 Q heads
# per core). Every core sees all 8 sequences, so the compiled schedule
# (derived from page_tables/context_lens, identical across cores) is SPMD.
#
# Orientation: scores are computed TRANSPOSED (S^T[t, sg] per 128-row
# t-block, K^T-stationary, Q^T-moving), so the exp (ACT) writes P^T directly
# in the layout the PV matmul consumes — no probability transposes anywhere.
# Masking folds into the exp's per-partition bias (host-precomputed -1e30
# rows for the partial page / 32-alignment gap / tail pad). The softmax
# denominator comes from an extra matmul with an all-ones stationary matrix,
# which leaves the per-sg denominator replicated across all 128 PSUM
# partitions — the normalization is then a single fused
# (OUT^T * 1/denom -> bf16) DVE pass, transposed back to [sg, d] by one
# DMA-xbar call per sequence.
#
# The block-causal mask for new tokens reduces (with sg = s*4+g ordering) to
# a suffix of valid sg columns per t-block (plus a small intra-block
# staircase zeroed on the bf16 P^T), so invalid regions are simply never
# computed.

import sys

sys.path.insert(0, "/opt/trn_rl_repo")

import ml_dtypes
import numpy as np

B = 8
S = 256
NUM_HEADS = 32
NUM_KV_HEADS = 8
G = NUM_HEADS // NUM_KV_HEADS  # 4
HD = 128
PAGE = 16
BLOCK = 32
MAX_PAGES = 128
C = MAX_PAGES * PAGE  # 2048
SCALE = 0.08838834764831845
SG = S * G  # 1024 q rows per (seq, kv head)
TMAX = C + S + 32  # worst-case padded length
NTBMAX = (TMAX + 127) // 128
NQT = SG // 128  # 8 q-tiles per seq

NEG = -1e30


def _schedule(page_tables: np.ndarray, context_lens: np.ndarray):
    """Per-seq schedule baked into the compiled kernel (same on all cores)."""
    seqs = []
    for b in range(B):
        ctx = int(context_lens[b])
        npg = (ctx + PAGE - 1) // PAGE
        ctxp = npg * PAGE
        ctxp32 = ((ctxp + 31) // 32) * 32  # 32-align the new-token region
        pages = [int(p) for p in page_tables[b, :npg]]
        runs = []  # maximal consecutive-page runs -> [start_page, num_pages]
        for p in pages:
            if runs and runs[-1][0] + runs[-1][1] == p:
                runs[-1][1] += 1
            else:
                runs.append([p, 1])
        ttot = ctxp32 + S
        ntb = (ttot + 127) // 128
        tq = [ctxp32 + BLOCK * (i + 1) for i in range(NQT)]
        # first valid q-tile per t-block (valid sg columns = suffix)
        qmin = [next(i for i in range(NQT) if tq[i] > tb * 128) for tb in range(ntb)]
        seqs.append(
            dict(
                ctx=ctx,
                ctxp=ctxp,
                ctxp32=ctxp32,
                runs=runs,
                ttot=ttot,
                ntb=ntb,
                tq=tq,
                qmin=qmin,
            )
        )
    return seqs


def _masks(seqs):
    """Host-precomputed per-partition exp bias: [B, 128, NTBMAX] fp32.
    mask[b, p, tb] is added (post-scale) to scores of t-row tb*128+p:
    0 for valid rows, -1e30 for masked rows (partial page, 32-align gap,
    padded tail)."""
    m = np.zeros((B, 128, NTBMAX), np.float32)
    for b, sq in enumerate(seqs):
        valid = np.zeros((NTBMAX * 128,), bool)
        valid[: sq["ttot"]] = True
        valid[sq["ctx"] : sq["ctxp32"]] = False  # partial page + gap
        m[b][~valid.reshape(NTBMAX, 128).T] = NEG
    return m


def _build(nc, seqs):
    import concourse.mybir as mybir
    import concourse.tile as tile

    bf16 = mybir.dt.bfloat16
    f32 = mybir.dt.float32

    qh = nc.dram_tensor("qh", [B * S, G * HD], bf16, kind="ExternalInput").ap()
    kh = nc.dram_tensor("kh", [B * S, HD], bf16, kind="ExternalInput").ap()
    vh = nc.dram_tensor("vh", [B * S, HD], bf16, kind="ExternalInput").ap()
    kch = nc.dram_tensor("kch", [MAX_PAGES * B * PAGE, HD], bf16, kind="ExternalInput").ap()
    vch = nc.dram_tensor("vch", [MAX_PAGES * B * PAGE, HD], bf16, kind="ExternalInput").ap()
    mh = nc.dram_tensor("mh", [B, 128, NTBMAX], f32, kind="ExternalInput").ap()
    zz = nc.dram_tensor("zz", [32, HD], bf16, kind="ExternalInput").ap()
    outh = nc.dram_tensor("outh", [B * S, G * HD], f32, kind="ExternalOutput").ap()

    # q viewed per seq as [sg=(s,g), d]; contiguous because each q row holds
    # the 4 grouped heads back to back. Same layout for the output.
    qv = qh.rearrange("(b s) (g d) -> b (s g) d", b=B, d=HD)
    outv = outh.rearrange("(b s) (g d) -> b (s g) d", b=B, d=HD)

    with tile.TileContext(nc) as tc:
        with (
            tc.tile_pool(name="cst", bufs=1) as const_pool,
            tc.tile_pool(name="kt", bufs=3) as kt_pool,
            tc.tile_pool(name="vt", bufs=3) as v_pool,
            tc.tile_pool(name="qt", bufs=3) as qt_pool,
            tc.tile_pool(name="pt", bufs=2) as pt_pool,
            tc.tile_pool(name="mk", bufs=3) as mask_pool,
            tc.tile_pool(name="ot", bufs=2) as out_pool,
            tc.tile_pool(name="ps_s", bufs=2, space="PSUM") as psum_s,
            tc.tile_pool(name="ps_o", bufs=1, space="PSUM") as psum_o,
            tc.tile_pool(name="ps_d", bufs=1, space="PSUM") as psum_d,
        ):
            ones_t = const_pool.tile([128, 128], bf16)
            nc.vector.memset(ones_t, 1.0)

            tiles = {}

            def emit_loads(b):
                sq = seqs[b]
                ctx, ctxp, ctxp32 = sq["ctx"], sq["ctxp"], sq["ctxp32"]
                ttot, ntb = sq["ttot"], sq["ntb"]

                # K^T via transposed loads (one call per page-run)
                kt = kt_pool.tile([128, NTBMAX * 128], bf16, tag="kt")
                col = 0
                for start, n in sq["runs"]:
                    nc.sync.dma_start_transpose(
                        kt[:, col : col + n * PAGE],
                        kch[start * PAGE : (start + n) * PAGE, :],
                    )
                    col += n * PAGE
                assert col == ctxp
                if ctxp32 > ctxp:  # 32-align gap: zero K columns
                    nc.vector.memset(kt[:, ctxp:ctxp32], 0.0)
                nc.sync.dma_start_transpose(
                    kt[:, ctxp32 : ctxp32 + S], kh[b * S : (b + 1) * S, :]
                )
                if ntb * 128 > ttot:  # zero padded tail columns
                    nc.vector.memset(kt[:, ttot : ntb * 128], 0.0)

                # V natural [t%128, tb, d]; big rearranged DMAs
                vt = v_pool.tile([128, NTBMAX, HD], bf16, tag="vt")
                if ttot % 128:
                    # zero last block before loads (NaN-safe padded tail)
                    nc.vector.memset(vt[:, ntb - 1, :], 0.0)

                def load_v_rows(t0, nrows, src, src_row0):
                    while nrows > 0 and t0 % 128:
                        seg = min(nrows, 128 - t0 % 128)
                        nc.gpsimd.dma_start(
                            vt[t0 % 128 : t0 % 128 + seg, t0 // 128, :],
                            src[src_row0 : src_row0 + seg, :],
                        )
                        t0 += seg
                        src_row0 += seg
                        nrows -= seg
                    nfull = (nrows // 128) * 128
                    if nfull:
                        nc.gpsimd.dma_start(
                            vt[:, t0 // 128 : t0 // 128 + nfull // 128, :],
                            src[src_row0 : src_row0 + nfull, :].rearrange(
                                "(tb p) d -> p tb d", p=128
                            ),
                        )
                        t0 += nfull
                        src_row0 += nfull
                        nrows -= nfull
                    if nrows:
                        nc.gpsimd.dma_start(
                            vt[:nrows, t0 // 128, :],
                            src[src_row0 : src_row0 + nrows, :],
                        )

                col = 0
                for start, n in sq["runs"]:
                    load_v_rows(col, n * PAGE, vch, start * PAGE)
                    col += n * PAGE
                if ctxp32 > ctxp and ctxp // 128 != ntb - 1:
                    # NaN-safe zeros for the gap rows
                    load_v_rows(ctxp, ctxp32 - ctxp, zz, 0)
                load_v_rows(ctxp32, S, vh, b * S)

                # Q^T via one transposed load
                qt = qt_pool.tile([128, SG], bf16, tag="qt")
                nc.sync.dma_start_transpose(qt, qv[b])

                # exp bias mask [128, ntb]
                mask_sb = mask_pool.tile([128, NTBMAX], f32, tag="mk")
                nc.sync.dma_start(mask_sb, mh[b])
                tiles[b] = (kt, vt, qt, mask_sb)

            def emit_compute(b):
                sq = seqs[b]
                ctxp32, ttot, ntb = sq["ctxp32"], sq["ttot"], sq["ntb"]
                tq, qmin = sq["tq"], sq["qmin"]
                kt, vt, qt, mask_sb = tiles[b]

                # per t-block: S^T = K^T.T @ Q^T -> exp -> P^T
                ptt = pt_pool.tile([128, NTBMAX, SG], bf16, tag="pt")
                for tb in range(ntb):
                    qm = qmin[tb]
                    s_ps = psum_s.tile([128, SG], f32, tag="s")
                    for c0, c1 in ((qm * 128, 512), (max(512, qm * 128), SG)):
                        if c0 >= c1:
                            continue
                        nc.tensor.matmul(
                            s_ps[:, c0:c1],
                            lhsT=kt[:, tb * 128 : (tb + 1) * 128],
                            rhs=qt[:, c0:c1],
                            start=True,
                            stop=True,
                        )
                    nc.scalar.activation(
                        out=ptt[:, tb, qm * 128 :],
                        in_=s_ps[:, qm * 128 :],
                        func=mybir.ActivationFunctionType.Exp,
                        scale=SCALE,
                        bias=mask_sb[:, tb : tb + 1],
                    )
                    # staircase: zero P^T rows of new-token blocks for
                    # earlier q-tiles inside this t-block's suffix
                    for r0 in range(0, 128, 32):
                        t0 = tb * 128 + r0
                        if t0 < ctxp32 or t0 >= ttot:
                            continue
                        blk = (t0 - ctxp32) // 32
                        if blk > qm:
                            nc.vector.memset(
                                ptt[r0 : r0 + 32, tb, qm * 128 : blk * 128], 0.0
                            )

                # PV: OUT^T[d, sg] += V[tb] (stationary) @ P^T[tb]
                outt = psum_o.tile([128, SG], f32, tag="outt")
                chunks = ((0, 4), (4, 8))
                last_tb = [0, 0]
                for tb in range(ntb):
                    for ci, (g0, g1) in enumerate(chunks):
                        if max(qmin[tb], g0) < g1:
                            last_tb[ci] = tb
                for tb in range(ntb):
                    for ci, (g0, g1) in enumerate(chunks):
                        lo = max(qmin[tb], g0)
                        if lo >= g1:
                            continue
                        nc.tensor.matmul(
                            outt[:, lo * 128 : g1 * 128],
                            lhsT=vt[:, tb, :],
                            rhs=ptt[:, tb, lo * 128 : g1 * 128],
                            start=(tb == 0),
                            stop=(tb == last_tb[ci]),
                        )

                # denominators: ones-matmul -> replicated column sums
                dent = psum_d.tile([128, SG], f32, tag="dent")
                for tb in range(ntb):
                    for ci, (g0, g1) in enumerate(chunks):
                        lo = max(qmin[tb], g0)
                        if lo >= g1:
                            continue
                        nc.tensor.matmul(
                            dent[:, lo * 128 : g1 * 128],
                            lhsT=ones_t,
                            rhs=ptt[:, tb, lo * 128 : g1 * 128],
                            start=(tb == 0),
                            stop=(tb == last_tb[ci]),
                        )
                tiles[b] = (outt, dent)

            def emit_endgame(b):
                outt, dent = tiles.pop(b)
                # OUT^T * (1/denom) -> bf16 -> transpose -> fp32 -> HBM
                invt = out_pool.tile([128, SG], f32, tag="invt")
                nc.vector.reciprocal_approx_fast(invt, dent)
                outt_sb = out_pool.tile([128, SG], bf16, tag="otsb")
                nc.vector.tensor_mul(outt_sb, outt, invt)
                out_sb = out_pool.tile([128, NQT, HD], bf16, tag="osb")
                nc.sync.dma_start_transpose(out_sb, outt_sb)
                of = out_pool.tile([128, NQT, HD], f32, tag="of")
                nc.vector.tensor_copy(of, out_sb)
                nc.gpsimd.dma_start(
                    outv[b].rearrange("(i p) d -> p i d", p=128), of
                )

            # software-pipelined emission: the in-order SP/Pool sequencers
            # must issue seq b+2's loads before blocking on seq b's endgame
            emit_loads(0)
            emit_loads(1)
            for b in range(B):
                emit_compute(b)
                if b + 2 < B:
                    emit_loads(b + 2)
                emit_endgame(b)
    return nc


def _compile(seqs):
    import concourse.bacc as bacc

    nc = bacc.Bacc(
        "TRN2",
        target_bir_lowering=False,
        debug=False,
        enable_asserts=False,
        num_devices=8,
    )
    _build(nc, seqs)
    nc.compile()
    return nc


def kernel(q, k, v, k_cache, v_cache, page_tables, context_lens, page_size, block_size, **_):
    from concourse import bass_utils

    q = np.asarray(q)
    k = np.asarray(k)
    v = np.asarray(v)
    k_cache = np.asarray(k_cache)
    v_cache = np.asarray(v_cache)
    page_tables = np.asarray(page_tables)
    context_lens = np.asarray(context_lens)
    assert int(page_size) == PAGE and int(block_size) == BLOCK
    assert q.shape == (B * S, NUM_HEADS * HD)
    assert page_tables.shape == (B, MAX_PAGES)

    seqs = _schedule(page_tables, context_lens)
    nc = _compile(seqs)

    bf = ml_dtypes.bfloat16
    masks = _masks(seqs)
    kcv = k_cache.reshape(MAX_PAGES * B * PAGE, NUM_KV_HEADS, HD)
    vcv = v_cache.reshape(MAX_PAGES * B * PAGE, NUM_KV_HEADS, HD)
    zz = np.zeros((32, HD), bf)
    in_maps = []
    for n in range(NUM_KV_HEADS):
        in_maps.append(
            {
                "qh": np.ascontiguousarray(
                    q[:, n * G * HD : (n + 1) * G * HD]
                ).astype(bf),
                "kh": np.ascontiguousarray(k[:, n * HD : (n + 1) * HD]).astype(bf),
                "vh": np.ascontiguousarray(v[:, n * HD : (n + 1) * HD]).astype(bf),
                "kch": np.ascontiguousarray(kcv[:, n, :]).astype(bf),
                "vch": np.ascontiguousarray(vcv[:, n, :]).astype(bf),
                "mh": masks,
                "zz": zz,
            }
        )

    res = bass_utils.run_bass_kernel_spmd(nc, in_maps, core_ids=list(range(8)))
    global _last_results
    _last_results = res
    out = np.concatenate([r["outh"] for r in res.results], axis=1)
    return out.astype(np.float32)


_last_results = None
